# revision 10
# baseline (speedup 1.0000x reference)
"""ChannelAttention kernel for Trainium2 (Bass/Tile), 8-core SPMD.

Reference (per sample b, xf = x[b] as [C=256, N=16384]):
    F  = W_f @ xf                      [50, N]
    S  = softmax(F @ xf^T, axis=C)     [50, 256]
    E  = S^T @ F ; out = W_beta @ E + xf

Key algebraic restructure: out = (W_beta @ S^T) @ F + x = M @ F + x where
M = W_beta @ S^T is a tiny [256, 50] matrix computed once per sample after
softmax — the big E tensor is never materialized.

Sharding: 8 cores = 4 samples x 2 spatial halves (x[b][:, h*8192:(h+1)*8192]).
The only cross-core coupling is the S contraction over N: partial S per
core, AllReduce within pairs [[0,1],[2,3],[4,5],[6,7]] (51 KB), then local.

Per-core dataflow:
  phase 1: F = W_f x; PE-transpose x and F tiles (n-on-partition) and
           accumulate partial S = F x^T in one PSUM bank.
  phase 2: AllReduce S, softmax over the free axis, M^T = S @ W_beta^T.
  phase 3: out = M F + x (residual via DVE add in fp32 mode, or via an
           identity matmul in fp32r mode); DMA out.

Two precision modes (CA_MODE env: "fast" | "safe", default "safe"):
  fast: every matmul in float32r (full PE rate, TF32-like ~7e-4 operand
        rounding). The S logits pick up ~0.1 noise over the 16k-term
        contraction.
  safe: the F and S matmuls (the softmax-logit path) run in true fp32
        (4 cycles/row); the post-softmax path stays fp32r where rounding
        is provably harmless.

n_iters > 1 repeats the whole dataflow (including DMAs and the collective)
inside one NEFF — used by test.py to measure per-iteration HW time by
differencing, since NTFF profiling is unavailable under axon.
"""

import os
import numpy as np
from contextlib import ExitStack

import concourse.bass as bass
import concourse.tile as tile
from concourse import mybir
from concourse.bass_utils import run_bass_kernel_spmd
from concourse.masks import make_identity

B, C, O = 4, 256, 50
N = 128 * 128            # 16384 spatial positions
NCORES = 8
NH = N // 2              # 8192 per core
NT = 512                 # matmul n-tile
NSUB = 128               # transpose / S sub-tile
XG = 2048                # x DMA group (1 MiB per chunk DMA)
F32 = mybir.dt.float32
F32R = mybir.dt.float32r
ActF = mybir.ActivationFunctionType

_CACHE: dict = {}
last_results = None  # exposes BassKernelResults to test.py

# This walrus build rejects instructions carrying more than one embedded
# semaphore wait ("Too many sync wait commands" in setupSyncWait). After
# Tile finishes sem assignment, hoist excess waits onto InstNoOp
# instructions inserted before the offender on the same engine — engine
# program order makes the split semantically identical.
_MAX_WAITS = 1


def _split_multiwait(nc) -> int:
    n_nops = 0
    for fn in nc.m.functions:
        for blk in fn.blocks:
            out = []
            changed = False
            for inst in list(blk.instructions):
                si = inst.sync_info
                waits = list(si.on_wait) if si is not None and si.on_wait else []
                if len(waits) > _MAX_WAITS:
                    keep = waits[-_MAX_WAITS:]
                    hoist = waits[:-_MAX_WAITS]
                    for i in range(0, len(hoist), _MAX_WAITS):
                        nop = mybir.InstNoOp(name=f"I-waitnop-{n_nops}")
                        n_nops += 1
                        nop.engine = inst.engine
                        nop.sync_info = mybir.SyncInfo(
                            on_wait=hoist[i:i + _MAX_WAITS], on_update=[]
                        )
                        out.append(nop)
                    changed = True
                    inst.sync_info = mybir.SyncInfo(
                        on_wait=keep,
                        on_update=list(si.on_update) if si.on_update else [],
                    )
                out.append(inst)
            if changed:
                blk.instructions = out
    return n_nops


def _build_nc(fast: bool, n_iters: int = 1) -> bass.Bass:
    nc = bass.Bass(num_devices=NCORES)

    xs = nc.dram_tensor("xs", [2, 128, NH], F32, kind="ExternalInput")
    wft = nc.dram_tensor("wft", [2, 128, O], F32, kind="ExternalInput")
    wbt = nc.dram_tensor("wbt", [2, 128, C], F32, kind="ExternalInput")
    out = nc.dram_tensor("out", [2, 128, NH], F32, kind="ExternalOutput")

    n_tiles = NH // NT            # 16
    n_groups = NH // XG           # 4 DMA groups per c-chunk
    subs = NT // NSUB             # 4 sub-tiles per n-tile
    XDT = F32R if fast else F32   # dtype of the softmax-logit path

    with tile.TileContext(nc) as tc, ExitStack() as ctx:
        const = ctx.enter_context(tc.tile_pool(name="const", bufs=1))
        xpool = ctx.enter_context(tc.tile_pool(name="x", bufs=1))
        fpool = ctx.enter_context(tc.tile_pool(name="f", bufs=1))
        stage = ctx.enter_context(tc.tile_pool(name="stage", bufs=4))
        spool = ctx.enter_context(tc.tile_pool(name="smax", bufs=1))
        opool = ctx.enter_context(tc.tile_pool(name="o", bufs=4))
        dram = ctx.enter_context(tc.tile_pool(name="dram", bufs=1, space="DRAM"))

        # weights first (tiny), then x loads can stream
        ident = const.tile([128, 128], F32, tag="ident")
        wft_sb = []
        wbt_sb = []
        for ci in range(2):
            t = const.tile([128, O], XDT, tag=f"wft{ci}")
            (nc.gpsimd if fast else nc.sync).dma_start(t[:], wft[ci])
            wft_sb.append(t)
            t = const.tile([128, C], F32R, tag=f"wbt{ci}")
            nc.gpsimd.dma_start(t[:], wbt[ci])
            wbt_sb.append(t)
        make_identity(nc, ident[:])
        if fast:
            ident_r = const.tile([128, 128], F32R, tag="ident_r")
            nc.vector.tensor_copy(ident_r[:], ident[:])

        def one_iter(it: int):
            # resident x: 2 c-chunks x 4 groups of [128, 2048]
            x_sb = [[None] * n_groups for _ in range(2)]
            for g in range(n_groups):
                for ci in range(2):
                    t = xpool.tile([128, XG], XDT, tag=f"x_{ci}_{g}")
                    (nc.gpsimd if fast else nc.sync).dma_start(
                        t[:], xs[ci, :, g * XG:(g + 1) * XG]
                    )
                    x_sb[ci][g] = t

            def xslice(ci, n0, w, as_f32=False):
                g, loc = divmod(n0, XG)
                assert loc + w <= XG
                ap = x_sb[ci][g][:, loc:loc + w]
                return ap.bitcast(F32) if (as_f32 and fast) else ap

            f_sb = fpool.tile([O, NH], XDT, tag="F")
            if fast:
                f_rhs = f_sb          # fp32r already
            else:
                f_rhs = fpool.tile([O, NH], F32R, tag="Fr")

            # ---- phase 1: F, x^T, partial S ----
            with tc.tile_pool(name=f"psS{it}", bufs=1, space="PSUM") as psS:
                s_ps = psS.tile([O, C], F32, tag="S")
                with tc.tile_pool(name=f"psF{it}", bufs=2, space="PSUM") as psF, \
                     tc.tile_pool(name=f"psT{it}", bufs=2, space="PSUM") as psT, \
                     tc.tile_pool(name=f"psFT{it}", bufs=2, space="PSUM") as psFT:
                    for nt in range(n_tiles):
                        n0 = nt * NT
                        f_ps = psF.tile([O, NT], F32, tag="f_ps")
                        for ci in range(2):
                            nc.tensor.matmul(
                                f_ps[:],
                                wft_sb[ci][:],
                                xslice(ci, n0, NT),
                                start=(ci == 0),
                                stop=(ci == 1),
                            )
                        nc.scalar.activation(
                            f_sb[:, n0:n0 + NT], f_ps[:], ActF.Copy
                        )
                        if not fast:
                            nc.scalar.activation(
                                f_rhs[:, n0:n0 + NT], f_ps[:], ActF.Copy
                            )

                        # x^T: 8 transposes -> 2 merged PSUM banks -> 2 copies
                        # layout [128, 512] = [s | s+1] x [ci0 | ci1]
                        xT_sb = []
                        for half in range(2):
                            tr_ps = psT.tile([128, 2, C], F32, tag="tr")
                            for s2 in range(2):
                                sn0 = n0 + (half * 2 + s2) * NSUB
                                for ci in range(2):
                                    nc.tensor.transpose(
                                        tr_ps[:, s2,
                                              ci * 128:(ci + 1) * 128],
                                        xslice(ci, sn0, NSUB, as_f32=True),
                                        ident[:],
                                    )
                            xT = stage.tile([128, 2, C], XDT, tag="xT")
                            nc.vector.tensor_copy(xT[:], tr_ps[:])
                            xT_sb.append(xT)

                        # F^T: 4 transposes -> 1 merged PSUM tile -> 1 copy
                        ftr_ps = psFT.tile([128, subs, O], F32, tag="ftr")
                        for s in range(subs):
                            sn0 = n0 + s * NSUB
                            fsrc = f_sb[:, sn0:sn0 + NSUB]
                            nc.tensor.transpose(
                                ftr_ps[:, s],
                                fsrc.bitcast(F32) if fast else fsrc,
                                ident[:O, :O],
                            )
                        fT = stage.tile([128, subs, O], XDT, tag="fT")
                        nc.vector.tensor_copy(fT[:], ftr_ps[:])

                        for s in range(subs):
                            idx = nt * subs + s
                            nc.tensor.matmul(
                                s_ps[:],
                                fT[:, s],
                                xT_sb[s // 2][:, s % 2],
                                start=(idx == 0),
                                stop=(idx == n_tiles * subs - 1),
                            )

                # ---- phase 2: AllReduce partial S + softmax + M ----
                s_part = spool.tile([O, C], F32, tag="s_part")
                nc.vector.tensor_copy(s_part[:], s_ps[:])

            cc_in = dram.tile([O, C], F32, tag="cc_in")
            cc_out = dram.tile([O, C], F32, tag="cc_out")
            nc.sync.dma_start(cc_in[:], s_part[:])
            nc.gpsimd.collective_compute(
                "AllReduce",
                mybir.AluOpType.add,
                replica_groups=[[0, 1], [2, 3], [4, 5], [6, 7]],
                ins=[cc_in.opt()],
                outs=[cc_out.opt()],
            )
            s_full = spool.tile([O, C], F32, tag="s_full")
            nc.sync.dma_start(s_full[:], cc_out[:])

            mx = spool.tile([O, 1], F32, tag="mx")
            nc.vector.tensor_reduce(
                mx[:], s_full[:], axis=mybir.AxisListType.X,
                op=mybir.AluOpType.max,
            )
            nmx = spool.tile([O, 1], F32, tag="nmx")
            nc.vector.tensor_scalar_mul(nmx[:], mx[:], -1.0)
            p_exp = spool.tile([O, C], F32, tag="p_exp")
            ssum = spool.tile([O, 1], F32, tag="ssum")
            nc.scalar.activation(
                p_exp[:], s_full[:], ActF.Exp, bias=nmx[:], accum_out=ssum[:]
            )
            rsum = spool.tile([O, 1], F32, tag="rsum")
            nc.vector.reciprocal(rsum[:], ssum[:])
            p_norm = spool.tile([O, C], F32, tag="p_norm")
            nc.vector.tensor_scalar_mul(p_norm[:], p_exp[:], rsum[:])

            # M^T = S @ W_beta^T  [50, 256]
            mT_sb = spool.tile([O, C], F32R, tag="mT")
            with tc.tile_pool(name=f"psM{it}", bufs=1, space="PSUM") as psM:
                st_sb = []
                for ci in range(2):
                    st_ps = psM.tile([128, O], F32, tag=f"st{ci}")
                    nc.tensor.transpose(
                        st_ps[:], p_norm[:, ci * 128:(ci + 1) * 128],
                        ident[:O, :O],
                    )
                    t = spool.tile([128, O], F32R, tag=f"st_sb{ci}")
                    nc.vector.tensor_copy(t[:], st_ps[:])
                    st_sb.append(t)
                m_ps = psM.tile([O, C], F32, tag="m")
                for ci in range(2):
                    nc.tensor.matmul(
                        m_ps[:],
                        st_sb[ci][:],
                        wbt_sb[ci][:],
                        start=(ci == 0),
                        stop=(ci == 1),
                    )
                nc.vector.tensor_copy(mT_sb[:], m_ps[:])

            # ---- phase 3: out = M F + x, two n-tiles per store DMA ----
            with tc.tile_pool(name=f"psO{it}", bufs=6, space="PSUM") as psO:
                for np2 in range(n_tiles // 2):
                    for d in range(2):
                        o_sb = opool.tile([128, 2 * NT], F32, tag="o_sb")
                        for k in range(2):
                            nt = np2 * 2 + k
                            n0 = nt * NT
                            o_ps = psO.tile([128, NT], F32, tag="o_ps")
                            if fast:
                                nc.tensor.matmul(
                                    o_ps[:],
                                    mT_sb[:, d * 128:(d + 1) * 128],
                                    f_rhs[:, n0:n0 + NT],
                                    start=True,
                                    stop=False,
                                )
                                nc.tensor.matmul(
                                    o_ps[:],
                                    ident_r[:],
                                    xslice(d, n0, NT),
                                    start=False,
                                    stop=True,
                                )
                                eng = nc.scalar if nt % 2 == 0 else nc.vector
                                if nt % 2 == 0:
                                    nc.scalar.activation(
                                        o_sb[:, k * NT:(k + 1) * NT],
                                        o_ps[:], ActF.Copy,
                                    )
                                else:
                                    nc.vector.tensor_copy(
                                        o_sb[:, k * NT:(k + 1) * NT], o_ps[:]
                                    )
                            elif nt % 2 == 0:
                                # residual via fp32 identity matmul, emitted
                                # FIRST so the scheduler can run it during
                                # the collective/softmax bubble (PE idle);
                                # the M-dependent matmul accumulates after.
                                nc.tensor.matmul(
                                    o_ps[:],
                                    ident[:],
                                    xslice(d, n0, NT),
                                    start=True,
                                    stop=False,
                                )
                                nc.tensor.matmul(
                                    o_ps[:],
                                    mT_sb[:, d * 128:(d + 1) * 128],
                                    f_rhs[:, n0:n0 + NT],
                                    start=False,
                                    stop=True,
                                )
                                nc.scalar.activation(
                                    o_sb[:, k * NT:(k + 1) * NT],
                                    o_ps[:], ActF.Copy,
                                )
                            else:
                                nc.tensor.matmul(
                                    o_ps[:],
                                    mT_sb[:, d * 128:(d + 1) * 128],
                                    f_rhs[:, n0:n0 + NT],
                                    start=True,
                                    stop=True,
                                )
                                nc.vector.tensor_add(
                                    o_sb[:, k * NT:(k + 1) * NT],
                                    o_ps[:], xslice(d, n0, NT),
                                )
                        n0 = np2 * 2 * NT
                        nc.sync.dma_start(
                            out[d, :, n0:n0 + 2 * NT], o_sb[:]
                        )

        for it in range(n_iters):
            one_iter(it)

    _split_multiwait(nc)
    return nc


def _get_nc(fast: bool, n_iters: int = 1):
    key = ("nc", fast, n_iters)
    if key not in _CACHE:
        _CACHE[key] = _build_nc(fast, n_iters)
    return _CACHE[key]


def _make_in_maps(x, W_f, W_beta):
    xf = np.ascontiguousarray(x.reshape(B, C, N), dtype=np.float32)
    wft = np.ascontiguousarray(W_f.T.reshape(2, 128, O), dtype=np.float32)
    wbt = np.ascontiguousarray(W_beta.T.reshape(2, 128, C), dtype=np.float32)
    in_maps = []
    for c in range(NCORES):
        b, h = divmod(c, 2)
        shard = np.ascontiguousarray(
            xf[b, :, h * NH:(h + 1) * NH].reshape(2, 128, NH)
        )
        in_maps.append({"xs": shard, "wft": wft, "wbt": wbt})
    return in_maps


def kernel(x: np.ndarray, W_f: np.ndarray, W_beta: np.ndarray) -> np.ndarray:
    global last_results
    fast = os.environ.get("CA_MODE", "safe") == "fast"
    nc = _get_nc(fast)

    in_maps = _make_in_maps(x, W_f, W_beta)
    res = run_bass_kernel_spmd(nc, in_maps, list(range(NCORES)))
    last_results = res

    outv = np.empty((B, C, N), dtype=np.float32)
    for c in range(NCORES):
        b, h = divmod(c, 2)
        outv[b, :, h * NH:(h + 1) * NH] = res.results[c]["out"].reshape(C, NH)
    return outv.reshape(B, C, 128, 128)


# revision 20
# speedup vs baseline: 1.0352x; 1.0352x over previous
"""ChannelAttention kernel for Trainium2 (Bass/Tile), 8-core SPMD.

Reference (per sample b, xf = x[b] as [C=256, N=16384]):
    F  = W_f @ xf                      [50, N]
    S  = softmax(F @ xf^T, axis=C)     [50, 256]
    E  = S^T @ F ; out = W_beta @ E + xf

Key algebraic restructure: out = (W_beta @ S^T) @ F + x = M @ F + x where
M = W_beta @ S^T is a tiny [256, 50] matrix computed once per sample after
softmax — the big E tensor is never materialized.

Sharding: 8 cores = 4 samples x 2 spatial halves (x[b][:, h*8192:(h+1)*8192]).
The only cross-core coupling is the S contraction over N: partial S per
core, AllReduce within pairs [[0,1],[2,3],[4,5],[6,7]] (51 KB), then local.

Per-core dataflow:
  phase 1: F = W_f x; PE-transpose x and F tiles (n-on-partition) and
           accumulate partial S = F x^T in one PSUM bank.
  phase 2: AllReduce S, softmax over the free axis, M^T = S @ W_beta^T.
  phase 3: out = M F + x (residual via DVE add in fp32 mode, or via an
           identity matmul in fp32r mode); DMA out.

Two precision modes (CA_MODE env: "fast" | "safe", default "safe"):
  fast: every matmul in float32r (full PE rate, TF32-like ~7e-4 operand
        rounding). The S logits pick up ~0.1 noise over the 16k-term
        contraction.
  safe: the F and S matmuls (the softmax-logit path) run in true fp32
        (4 cycles/row); the post-softmax path stays fp32r where rounding
        is provably harmless.

n_iters > 1 repeats the whole dataflow (including DMAs and the collective)
inside one NEFF — used by test.py to measure per-iteration HW time by
differencing, since NTFF profiling is unavailable under axon.
"""

import os
import numpy as np
from contextlib import ExitStack

import concourse.bass as bass
import concourse.tile as tile
from concourse import mybir
from concourse.bass_utils import run_bass_kernel_spmd
from concourse.masks import make_identity

B, C, O = 4, 256, 50
N = 128 * 128            # 16384 spatial positions
NCORES = 8
NH = N // 2              # 8192 per core
NT = 512                 # matmul n-tile
NSUB = 128               # transpose / S sub-tile
XG = 2048                # x DMA group (1 MiB per chunk DMA)
F32 = mybir.dt.float32
F32R = mybir.dt.float32r
ActF = mybir.ActivationFunctionType

_CACHE: dict = {}
last_results = None  # exposes BassKernelResults to test.py

# This walrus build rejects instructions carrying more than one embedded
# semaphore wait ("Too many sync wait commands" in setupSyncWait). After
# Tile finishes sem assignment, hoist excess waits onto InstNoOp
# instructions inserted before the offender on the same engine — engine
# program order makes the split semantically identical.
_MAX_WAITS = 1


def _split_multiwait(nc) -> int:
    n_nops = 0
    for fn in nc.m.functions:
        for blk in fn.blocks:
            out = []
            changed = False
            for inst in list(blk.instructions):
                si = inst.sync_info
                waits = list(si.on_wait) if si is not None and si.on_wait else []
                if len(waits) > _MAX_WAITS:
                    keep = waits[-_MAX_WAITS:]
                    hoist = waits[:-_MAX_WAITS]
                    for i in range(0, len(hoist), _MAX_WAITS):
                        nop = mybir.InstNoOp(name=f"I-waitnop-{n_nops}")
                        n_nops += 1
                        nop.engine = inst.engine
                        nop.sync_info = mybir.SyncInfo(
                            on_wait=hoist[i:i + _MAX_WAITS], on_update=[]
                        )
                        out.append(nop)
                    changed = True
                    inst.sync_info = mybir.SyncInfo(
                        on_wait=keep,
                        on_update=list(si.on_update) if si.on_update else [],
                    )
                out.append(inst)
            if changed:
                blk.instructions = out
    return n_nops


def _build_nc(fast: bool, n_iters: int = 1,
              skip_phase3: bool = False, skip_cc: bool = False) -> bass.Bass:
    """skip_* flags build ablated variants for phase-isolation timing on
    hardware (no NTFF profiler under axon); kernel() never sets them."""
    nc = bass.Bass(num_devices=NCORES)

    xs = nc.dram_tensor("xs", [2, 128, NH], F32, kind="ExternalInput")
    wft = nc.dram_tensor("wft", [2, 128, O], F32, kind="ExternalInput")
    wbt = nc.dram_tensor("wbt", [2, 128, C], F32, kind="ExternalInput")
    out = nc.dram_tensor("out", [2, 128, NH], F32, kind="ExternalOutput")

    n_tiles = NH // NT            # 16
    n_groups = NH // XG           # 4 DMA groups per c-chunk
    subs = NT // NSUB             # 4 sub-tiles per n-tile
    XDT = F32R if fast else F32   # dtype of the softmax-logit path

    with tile.TileContext(nc) as tc, ExitStack() as ctx:
        const = ctx.enter_context(tc.tile_pool(name="const", bufs=1))
        xpool = ctx.enter_context(tc.tile_pool(name="x", bufs=1))
        fpool = ctx.enter_context(tc.tile_pool(name="f", bufs=1))
        stage = ctx.enter_context(tc.tile_pool(name="stage", bufs=4))
        spool = ctx.enter_context(tc.tile_pool(name="smax", bufs=1))
        opool = ctx.enter_context(tc.tile_pool(name="o", bufs=3))
        dram = ctx.enter_context(tc.tile_pool(name="dram", bufs=1, space="DRAM"))

        # weights first (tiny), then x loads can stream
        ident = const.tile([128, 128], F32, tag="ident")
        wft_sb = []
        wbt_sb = []
        for ci in range(2):
            t = const.tile([128, O], XDT, tag=f"wft{ci}")
            (nc.gpsimd if fast else nc.sync).dma_start(t[:], wft[ci])
            wft_sb.append(t)
            t = const.tile([128, C], F32R, tag=f"wbt{ci}")
            nc.gpsimd.dma_start(t[:], wbt[ci])
            wbt_sb.append(t)
        make_identity(nc, ident[:])
        if fast:
            ident_r = const.tile([128, 128], F32R, tag="ident_r")
            nc.vector.tensor_copy(ident_r[:], ident[:])

        def one_iter(it: int):
            # resident x: 2 c-chunks x 4 groups of [128, 2048]; alternate
            # between the two physical HWDGE rings (SP and ACT) so transfer
            # completions overlap instead of serializing on one FIFO
            x_sb = [[None] * n_groups for _ in range(2)]
            for g in range(n_groups):
                for ci in range(2):
                    t = xpool.tile([128, XG], XDT, tag=f"x_{ci}_{g}")
                    if fast:
                        eng = nc.gpsimd
                    else:
                        eng = nc.sync if ci == 0 else nc.scalar
                    eng.dma_start(t[:], xs[ci, :, g * XG:(g + 1) * XG])
                    x_sb[ci][g] = t

            def xslice(ci, n0, w, as_f32=False):
                g, loc = divmod(n0, XG)
                assert loc + w <= XG
                ap = x_sb[ci][g][:, loc:loc + w]
                return ap.bitcast(F32) if (as_f32 and fast) else ap

            f_sb = fpool.tile([O, NH], XDT, tag="F")
            if fast:
                f_rhs = f_sb          # fp32r already
            else:
                f_rhs = fpool.tile([O, NH], F32R, tag="Fr")

            # ---- phase 1: F, x^T, partial S ----
            with tc.tile_pool(name=f"psS{it}", bufs=1, space="PSUM") as psS:
                s_ps = psS.tile([O, C], F32, tag="S")
                with tc.tile_pool(name=f"psF{it}", bufs=2, space="PSUM") as psF, \
                     tc.tile_pool(name=f"psT{it}", bufs=2, space="PSUM") as psT, \
                     tc.tile_pool(name=f"psFT{it}", bufs=2, space="PSUM") as psFT:
                    for nt in range(n_tiles):
                        n0 = nt * NT
                        f_ps = psF.tile([O, NT], F32, tag="f_ps")
                        for ci in range(2):
                            nc.tensor.matmul(
                                f_ps[:],
                                wft_sb[ci][:],
                                xslice(ci, n0, NT),
                                start=(ci == 0),
                                stop=(ci == 1),
                            )
                        nc.scalar.activation(
                            f_sb[:, n0:n0 + NT], f_ps[:], ActF.Copy
                        )
                        if not fast:
                            nc.scalar.activation(
                                f_rhs[:, n0:n0 + NT], f_ps[:], ActF.Copy
                            )

                        # x^T: 8 transposes -> 2 merged PSUM banks -> 2 copies
                        # layout [128, 512] = [s | s+1] x [ci0 | ci1]
                        xT_sb = []
                        for half in range(2):
                            tr_ps = psT.tile([128, 2, C], F32, tag="tr")
                            for s2 in range(2):
                                sn0 = n0 + (half * 2 + s2) * NSUB
                                for ci in range(2):
                                    nc.tensor.transpose(
                                        tr_ps[:, s2,
                                              ci * 128:(ci + 1) * 128],
                                        xslice(ci, sn0, NSUB, as_f32=True),
                                        ident[:],
                                    )
                            xT = stage.tile([128, 2, C], XDT, tag="xT")
                            nc.vector.tensor_copy(xT[:], tr_ps[:])
                            xT_sb.append(xT)

                        # F^T: 4 transposes -> 1 merged PSUM tile -> 1 copy
                        ftr_ps = psFT.tile([128, subs, O], F32, tag="ftr")
                        for s in range(subs):
                            sn0 = n0 + s * NSUB
                            fsrc = f_sb[:, sn0:sn0 + NSUB]
                            nc.tensor.transpose(
                                ftr_ps[:, s],
                                fsrc.bitcast(F32) if fast else fsrc,
                                ident[:O, :O],
                            )
                        fT = stage.tile([128, subs, O], XDT, tag="fT")
                        nc.vector.tensor_copy(fT[:], ftr_ps[:])

                        for s in range(subs):
                            idx = nt * subs + s
                            nc.tensor.matmul(
                                s_ps[:],
                                fT[:, s],
                                xT_sb[s // 2][:, s % 2],
                                start=(idx == 0),
                                stop=(idx == n_tiles * subs - 1),
                            )

                # ---- phase 2: AllReduce partial S + softmax + M ----
                s_part = spool.tile([O, C], F32, tag="s_part")
                nc.vector.tensor_copy(s_part[:], s_ps[:])

            cc_in = dram.tile([O, C], F32, tag="cc_in")
            cc_out = dram.tile([O, C], F32, tag="cc_out")
            nc.sync.dma_start(cc_in[:], s_part[:])
            if skip_cc:
                nc.sync.dma_start(cc_out[:], cc_in[:])
            else:
                nc.gpsimd.collective_compute(
                    "AllReduce",
                    mybir.AluOpType.add,
                    replica_groups=[[0, 1], [2, 3], [4, 5], [6, 7]],
                    ins=[cc_in.opt()],
                    outs=[cc_out.opt()],
                )
            s_full = spool.tile([O, C], F32, tag="s_full")
            nc.sync.dma_start(s_full[:], cc_out[:])

            mx = spool.tile([O, 1], F32, tag="mx")
            nc.vector.tensor_reduce(
                mx[:], s_full[:], axis=mybir.AxisListType.X,
                op=mybir.AluOpType.max,
            )
            nmx = spool.tile([O, 1], F32, tag="nmx")
            nc.vector.tensor_scalar_mul(nmx[:], mx[:], -1.0)
            p_exp = spool.tile([O, C], F32, tag="p_exp")
            ssum = spool.tile([O, 1], F32, tag="ssum")
            nc.scalar.activation(
                p_exp[:], s_full[:], ActF.Exp, bias=nmx[:], accum_out=ssum[:]
            )
            rsum = spool.tile([O, 1], F32, tag="rsum")
            nc.vector.reciprocal(rsum[:], ssum[:])
            p_norm = spool.tile([O, C], F32, tag="p_norm")
            nc.vector.tensor_scalar_mul(p_norm[:], p_exp[:], rsum[:])

            # ---- phase 3: out = M F + x, two n-tiles per store DMA ----
            # psO opens BEFORE psM so the PSUM stack gives psO banks that
            # don't wait on psM's release: the residual identity-matmuls
            # (which don't depend on M) can then fill o_ps banks during the
            # collective/softmax bubble. psM uses a single sequentially
            # reused bank (6 + 1 <= 8).
            with tc.tile_pool(name=f"psO{it}", bufs=6, space="PSUM") as psO, \
                 tc.tile_pool(name=f"psM{it}", bufs=1, space="PSUM") as psM:
                # M^T = S @ W_beta^T  [50, 256]
                mT_sb = spool.tile([O, C], F32R, tag="mT")
                st_sb = []
                for ci in range(2):
                    st_ps = psM.tile([128, O], F32, tag="m_seq")
                    nc.tensor.transpose(
                        st_ps[:], p_norm[:, ci * 128:(ci + 1) * 128],
                        ident[:O, :O],
                    )
                    t = spool.tile([128, O], F32R, tag=f"st_sb{ci}")
                    nc.vector.tensor_copy(t[:], st_ps[:])
                    st_sb.append(t)
                m_ps = psM.tile([O, C], F32, tag="m_seq")
                for ci in range(2):
                    nc.tensor.matmul(
                        m_ps[:],
                        st_sb[ci][:],
                        wbt_sb[ci][:],
                        start=(ci == 0),
                        stop=(ci == 1),
                    )
                nc.vector.tensor_copy(mT_sb[:], m_ps[:])

                if skip_phase3:
                    return
                for np4 in range(n_tiles // 4):
                    for d in range(2):
                        # 4 n-tiles per 1 MiB store; alternate HWDGE rings
                        o_sb = opool.tile([128, 4 * NT], F32, tag="o_sb")
                        for k in range(4):
                            nt = np4 * 4 + k
                            n0 = nt * NT
                            o_ps = psO.tile([128, NT], F32, tag="o_ps")
                            if fast:
                                nc.tensor.matmul(
                                    o_ps[:],
                                    mT_sb[:, d * 128:(d + 1) * 128],
                                    f_rhs[:, n0:n0 + NT],
                                    start=True,
                                    stop=False,
                                )
                                nc.tensor.matmul(
                                    o_ps[:],
                                    ident_r[:],
                                    xslice(d, n0, NT),
                                    start=False,
                                    stop=True,
                                )
                                eng = nc.scalar if nt % 2 == 0 else nc.vector
                                if nt % 2 == 0:
                                    nc.scalar.activation(
                                        o_sb[:, k * NT:(k + 1) * NT],
                                        o_ps[:], ActF.Copy,
                                    )
                                else:
                                    nc.vector.tensor_copy(
                                        o_sb[:, k * NT:(k + 1) * NT], o_ps[:]
                                    )
                            elif nt % 2 == 0:
                                # residual via fp32 identity matmul, emitted
                                # FIRST so the scheduler can run it during
                                # the collective/softmax bubble (PE idle);
                                # the M-dependent matmul accumulates after.
                                nc.tensor.matmul(
                                    o_ps[:],
                                    ident[:],
                                    xslice(d, n0, NT),
                                    start=True,
                                    stop=False,
                                )
                                nc.tensor.matmul(
                                    o_ps[:],
                                    mT_sb[:, d * 128:(d + 1) * 128],
                                    f_rhs[:, n0:n0 + NT],
                                    start=False,
                                    stop=True,
                                )
                                nc.scalar.activation(
                                    o_sb[:, k * NT:(k + 1) * NT],
                                    o_ps[:], ActF.Copy,
                                )
                            else:
                                nc.tensor.matmul(
                                    o_ps[:],
                                    mT_sb[:, d * 128:(d + 1) * 128],
                                    f_rhs[:, n0:n0 + NT],
                                    start=True,
                                    stop=True,
                                )
                                nc.vector.tensor_add(
                                    o_sb[:, k * NT:(k + 1) * NT],
                                    o_ps[:], xslice(d, n0, NT),
                                )
                        n0 = np4 * 4 * NT
                        (nc.sync if (np4 + d) % 2 == 0 else nc.scalar).dma_start(
                            out[d, :, n0:n0 + 4 * NT], o_sb[:]
                        )

        for it in range(n_iters):
            one_iter(it)

    _split_multiwait(nc)
    return nc


def _get_nc(fast: bool, n_iters: int = 1):
    key = ("nc", fast, n_iters)
    if key not in _CACHE:
        _CACHE[key] = _build_nc(fast, n_iters)
    return _CACHE[key]


def _make_in_maps(x, W_f, W_beta):
    xf = np.ascontiguousarray(x.reshape(B, C, N), dtype=np.float32)
    wft = np.ascontiguousarray(W_f.T.reshape(2, 128, O), dtype=np.float32)
    wbt = np.ascontiguousarray(W_beta.T.reshape(2, 128, C), dtype=np.float32)
    in_maps = []
    for c in range(NCORES):
        b, h = divmod(c, 2)
        shard = np.ascontiguousarray(
            xf[b, :, h * NH:(h + 1) * NH].reshape(2, 128, NH)
        )
        in_maps.append({"xs": shard, "wft": wft, "wbt": wbt})
    return in_maps


def kernel(x: np.ndarray, W_f: np.ndarray, W_beta: np.ndarray) -> np.ndarray:
    global last_results
    fast = os.environ.get("CA_MODE", "safe") == "fast"
    nc = _get_nc(fast)

    in_maps = _make_in_maps(x, W_f, W_beta)
    res = run_bass_kernel_spmd(nc, in_maps, list(range(NCORES)))
    last_results = res

    outv = np.empty((B, C, N), dtype=np.float32)
    for c in range(NCORES):
        b, h = divmod(c, 2)
        outv[b, :, h * NH:(h + 1) * NH] = res.results[c]["out"].reshape(C, NH)
    return outv.reshape(B, C, 128, 128)


# revision 24
# speedup vs baseline: 1.2835x; 1.2398x over previous
"""ChannelAttention kernel for Trainium2 (Bass/Tile), 8-core SPMD.

Reference (per sample b, xf = x[b] as [C=256, N=16384]):
    F  = W_f @ xf                      [50, N]
    S  = softmax(F @ xf^T, axis=C)     [50, 256]
    E  = S^T @ F ; out = W_beta @ E + xf

Key algebraic restructure: out = (W_beta @ S^T) @ F + x = M @ F + x where
M = W_beta @ S^T is a tiny [256, 50] matrix computed once per sample after
softmax — the big E tensor is never materialized.

Sharding: 8 cores = 4 samples x 2 spatial halves (x[b][:, h*8192:(h+1)*8192]).
The only cross-core coupling is the S contraction over N: partial S per
core, AllReduce within pairs [[0,1],[2,3],[4,5],[6,7]] (51 KB), then local.

Per-core dataflow:
  phase 1: F = W_f x; PE-transpose x and F tiles (n-on-partition) and
           accumulate partial S = F x^T in one PSUM bank.
  phase 2: AllReduce S, softmax over the free axis, M^T = S @ W_beta^T.
  phase 3: out = M F + x (residual via DVE add in fp32 mode, or via an
           identity matmul in fp32r mode); DMA out.

Two precision modes (CA_MODE env: "fast" | "safe", default "safe"):
  fast: every matmul in float32r (full PE rate, TF32-like ~7e-4 operand
        rounding). The S logits pick up ~0.1 noise over the 16k-term
        contraction.
  safe: the F and S matmuls (the softmax-logit path) run in true fp32
        (4 cycles/row); the post-softmax path stays fp32r where rounding
        is provably harmless.

n_iters > 1 repeats the whole dataflow (including DMAs and the collective)
inside one NEFF — used by test.py to measure per-iteration HW time by
differencing, since NTFF profiling is unavailable under axon.
"""

import os
import numpy as np
from contextlib import ExitStack

import concourse.bass as bass
import concourse.tile as tile
from concourse import mybir
from concourse.bass_utils import run_bass_kernel_spmd
from concourse.masks import make_identity

B, C, O = 4, 256, 50
N = 128 * 128            # 16384 spatial positions
NCORES = 8
NH = N // 2              # 8192 per core
NT = 512                 # matmul n-tile
NSUB = 128               # transpose / S sub-tile
XG = 2048                # x DMA group (1 MiB per chunk DMA)
F32 = mybir.dt.float32
F32R = mybir.dt.float32r
BF16 = mybir.dt.bfloat16
ActF = mybir.ActivationFunctionType

_CACHE: dict = {}
last_results = None  # exposes BassKernelResults to test.py

# This walrus build rejects instructions carrying more than one embedded
# semaphore wait ("Too many sync wait commands" in setupSyncWait). After
# Tile finishes sem assignment, hoist excess waits onto InstNoOp
# instructions inserted before the offender on the same engine — engine
# program order makes the split semantically identical.
_MAX_WAITS = 1


def _split_multiwait(nc) -> int:
    n_nops = 0
    for fn in nc.m.functions:
        for blk in fn.blocks:
            out = []
            changed = False
            for inst in list(blk.instructions):
                si = inst.sync_info
                waits = list(si.on_wait) if si is not None and si.on_wait else []
                if len(waits) > _MAX_WAITS:
                    keep = waits[-_MAX_WAITS:]
                    hoist = waits[:-_MAX_WAITS]
                    for i in range(0, len(hoist), _MAX_WAITS):
                        nop = mybir.InstNoOp(name=f"I-waitnop-{n_nops}")
                        n_nops += 1
                        nop.engine = inst.engine
                        nop.sync_info = mybir.SyncInfo(
                            on_wait=hoist[i:i + _MAX_WAITS], on_update=[]
                        )
                        out.append(nop)
                    changed = True
                    inst.sync_info = mybir.SyncInfo(
                        on_wait=keep,
                        on_update=list(si.on_update) if si.on_update else [],
                    )
                out.append(inst)
            if changed:
                blk.instructions = out
    return n_nops


def _build_nc(fast: bool, n_iters: int = 1,
              skip_phase3: bool = False, skip_cc: bool = False) -> bass.Bass:
    """skip_* flags build ablated variants for phase-isolation timing on
    hardware (no NTFF profiler under axon); kernel() never sets them."""
    nc = bass.Bass(num_devices=NCORES)

    xs = nc.dram_tensor("xs", [2, 128, NH], F32, kind="ExternalInput")
    wft = nc.dram_tensor("wft", [2, 128, O], F32, kind="ExternalInput")
    wbt = nc.dram_tensor("wbt", [2, 128, C], F32, kind="ExternalInput")
    out = nc.dram_tensor("out", [2, 128, NH], F32, kind="ExternalOutput")

    n_tiles = NH // NT            # 16
    n_groups = NH // XG           # 4 DMA groups per c-chunk
    subs = NT // NSUB             # 4 sub-tiles per n-tile
    XDT = F32R if fast else F32   # dtype of the softmax-logit path

    with tile.TileContext(nc) as tc, ExitStack() as ctx:
        const = ctx.enter_context(tc.tile_pool(name="const", bufs=1))
        xpool = ctx.enter_context(tc.tile_pool(name="x", bufs=1))
        fpool = ctx.enter_context(tc.tile_pool(name="f", bufs=1))
        stage = ctx.enter_context(tc.tile_pool(name="stage", bufs=4))
        spool = ctx.enter_context(tc.tile_pool(name="smax", bufs=1))
        opool = ctx.enter_context(tc.tile_pool(name="o", bufs=3))
        dram = ctx.enter_context(tc.tile_pool(name="dram", bufs=1, space="DRAM"))

        # weights first (tiny), then x loads can stream
        ident = const.tile([128, 128], F32, tag="ident")
        wft_sb = []
        wbt_sb = []
        for ci in range(2):
            t = const.tile([128, O], XDT, tag=f"wft{ci}")
            (nc.gpsimd if fast else nc.sync).dma_start(t[:], wft[ci])
            wft_sb.append(t)
            t = const.tile([128, C], F32R, tag=f"wbt{ci}")
            nc.gpsimd.dma_start(t[:], wbt[ci])
            wbt_sb.append(t)
        make_identity(nc, ident[:])
        if fast:
            ident_r = const.tile([128, 128], F32R, tag="ident_r")
            nc.vector.tensor_copy(ident_r[:], ident[:])

        def one_iter(it: int):
            # resident x: 2 c-chunks x 4 groups of [128, 2048]; alternate
            # between the two physical HWDGE rings (SP and ACT) so transfer
            # completions overlap instead of serializing on one FIFO
            x_sb = [[None] * n_groups for _ in range(2)]
            for g in range(n_groups):
                for ci in range(2):
                    t = xpool.tile([128, XG], XDT, tag=f"x_{ci}_{g}")
                    if fast:
                        eng = nc.gpsimd
                    else:
                        eng = nc.sync if ci == 0 else nc.scalar
                    eng.dma_start(t[:], xs[ci, :, g * XG:(g + 1) * XG])
                    x_sb[ci][g] = t

            def xslice(ci, n0, w, as_f32=False):
                g, loc = divmod(n0, XG)
                assert loc + w <= XG
                ap = x_sb[ci][g][:, loc:loc + w]
                return ap.bitcast(F32) if (as_f32 and fast) else ap

            f_sb = fpool.tile([O, NH], XDT, tag="F")
            if fast:
                f_rhs = f_sb          # fp32r already
            else:
                # bf16 copy for the post-softmax out-matmul: bf16 runs the
                # guaranteed-native 1 cycle/row PE path, and |out| error from
                # rounding F here is ~3e-4 of output scale (post-softmax,
                # no logit sensitivity)
                f_rhs = fpool.tile([O, NH], BF16, tag="Fr")

            # ---- phase 1: F, x^T, partial S ----
            with tc.tile_pool(name=f"psS{it}", bufs=1, space="PSUM") as psS:
                s_ps = psS.tile([O, C], F32, tag="S")
                with tc.tile_pool(name=f"psF{it}", bufs=2, space="PSUM") as psF, \
                     tc.tile_pool(name=f"psT{it}", bufs=2, space="PSUM") as psT, \
                     tc.tile_pool(name=f"psFT{it}", bufs=2, space="PSUM") as psFT:
                    for nt in range(n_tiles):
                        n0 = nt * NT
                        f_ps = psF.tile([O, NT], F32, tag="f_ps")
                        for ci in range(2):
                            nc.tensor.matmul(
                                f_ps[:],
                                wft_sb[ci][:],
                                xslice(ci, n0, NT),
                                start=(ci == 0),
                                stop=(ci == 1),
                            )
                        nc.scalar.activation(
                            f_sb[:, n0:n0 + NT], f_ps[:], ActF.Copy
                        )
                        if not fast:
                            nc.scalar.activation(
                                f_rhs[:, n0:n0 + NT], f_ps[:], ActF.Copy
                            )

                        # x^T: 8 transposes -> 2 merged PSUM banks -> 2 copies
                        # layout [128, 512] = [s | s+1] x [ci0 | ci1]
                        xT_sb = []
                        for half in range(2):
                            tr_ps = psT.tile([128, 2, C], F32, tag="tr")
                            for s2 in range(2):
                                sn0 = n0 + (half * 2 + s2) * NSUB
                                for ci in range(2):
                                    nc.tensor.transpose(
                                        tr_ps[:, s2,
                                              ci * 128:(ci + 1) * 128],
                                        xslice(ci, sn0, NSUB, as_f32=True),
                                        ident[:],
                                    )
                            xT = stage.tile([128, 2, C], XDT, tag="xT")
                            nc.vector.tensor_copy(xT[:], tr_ps[:])
                            xT_sb.append(xT)

                        # F^T: 4 transposes -> 1 merged PSUM tile -> 1 copy
                        ftr_ps = psFT.tile([128, subs, O], F32, tag="ftr")
                        for s in range(subs):
                            sn0 = n0 + s * NSUB
                            fsrc = f_sb[:, sn0:sn0 + NSUB]
                            nc.tensor.transpose(
                                ftr_ps[:, s],
                                fsrc.bitcast(F32) if fast else fsrc,
                                ident[:O, :O],
                            )
                        fT = stage.tile([128, subs, O], XDT, tag="fT")
                        nc.vector.tensor_copy(fT[:], ftr_ps[:])

                        for s in range(subs):
                            idx = nt * subs + s
                            nc.tensor.matmul(
                                s_ps[:],
                                fT[:, s],
                                xT_sb[s // 2][:, s % 2],
                                start=(idx == 0),
                                stop=(idx == n_tiles * subs - 1),
                            )

                # ---- phase 2: AllReduce partial S + softmax + M ----
                s_part = spool.tile([O, C], F32, tag="s_part")
                nc.vector.tensor_copy(s_part[:], s_ps[:])

            cc_in = dram.tile([O, C], F32, tag="cc_in")
            cc_out = dram.tile([O, C], F32, tag="cc_out")
            nc.sync.dma_start(cc_in[:], s_part[:])
            if skip_cc:
                nc.sync.dma_start(cc_out[:], cc_in[:])
            else:
                nc.gpsimd.collective_compute(
                    "AllReduce",
                    mybir.AluOpType.add,
                    replica_groups=[[0, 1], [2, 3], [4, 5], [6, 7]],
                    ins=[cc_in.opt()],
                    outs=[cc_out.opt()],
                )
            s_full = spool.tile([O, C], F32, tag="s_full")
            nc.sync.dma_start(s_full[:], cc_out[:])

            mx = spool.tile([O, 1], F32, tag="mx")
            nc.vector.tensor_reduce(
                mx[:], s_full[:], axis=mybir.AxisListType.X,
                op=mybir.AluOpType.max,
            )
            nmx = spool.tile([O, 1], F32, tag="nmx")
            nc.vector.tensor_scalar_mul(nmx[:], mx[:], -1.0)
            p_exp = spool.tile([O, C], F32, tag="p_exp")
            ssum = spool.tile([O, 1], F32, tag="ssum")
            nc.scalar.activation(
                p_exp[:], s_full[:], ActF.Exp, bias=nmx[:], accum_out=ssum[:]
            )
            rsum = spool.tile([O, 1], F32, tag="rsum")
            nc.vector.reciprocal(rsum[:], ssum[:])
            p_norm = spool.tile([O, C], F32, tag="p_norm")
            nc.vector.tensor_scalar_mul(p_norm[:], p_exp[:], rsum[:])

            # ---- phase 3: out = M F + x, two n-tiles per store DMA ----
            # psO opens BEFORE psM so the PSUM stack gives psO banks that
            # don't wait on psM's release: the residual identity-matmuls
            # (which don't depend on M) can then fill o_ps banks during the
            # collective/softmax bubble. psM uses a single sequentially
            # reused bank (6 + 1 <= 8).
            with tc.tile_pool(name=f"psO{it}", bufs=6, space="PSUM") as psO, \
                 tc.tile_pool(name=f"psM{it}", bufs=1, space="PSUM") as psM:
                # M^T = S @ W_beta^T  [50, 256]
                mT_sb = spool.tile([O, C], F32R if fast else BF16, tag="mT")
                st_sb = []
                for ci in range(2):
                    st_ps = psM.tile([128, O], F32, tag="m_seq")
                    nc.tensor.transpose(
                        st_ps[:], p_norm[:, ci * 128:(ci + 1) * 128],
                        ident[:O, :O],
                    )
                    t = spool.tile([128, O], F32R, tag=f"st_sb{ci}")
                    nc.vector.tensor_copy(t[:], st_ps[:])
                    st_sb.append(t)
                m_ps = psM.tile([O, C], F32, tag="m_seq")
                for ci in range(2):
                    nc.tensor.matmul(
                        m_ps[:],
                        st_sb[ci][:],
                        wbt_sb[ci][:],
                        start=(ci == 0),
                        stop=(ci == 1),
                    )
                nc.vector.tensor_copy(mT_sb[:], m_ps[:])

                if skip_phase3:
                    return
                for np4 in range(n_tiles // 4):
                    for d in range(2):
                        # 4 n-tiles per 1 MiB store; alternate HWDGE rings
                        o_sb = opool.tile([128, 4 * NT], F32, tag="o_sb")
                        for k in range(4):
                            nt = np4 * 4 + k
                            n0 = nt * NT
                            o_ps = psO.tile([128, NT], F32, tag="o_ps")
                            if fast:
                                nc.tensor.matmul(
                                    o_ps[:],
                                    mT_sb[:, d * 128:(d + 1) * 128],
                                    f_rhs[:, n0:n0 + NT],
                                    start=True,
                                    stop=False,
                                )
                                nc.tensor.matmul(
                                    o_ps[:],
                                    ident_r[:],
                                    xslice(d, n0, NT),
                                    start=False,
                                    stop=True,
                                )
                                eng = nc.scalar if nt % 2 == 0 else nc.vector
                                if nt % 2 == 0:
                                    nc.scalar.activation(
                                        o_sb[:, k * NT:(k + 1) * NT],
                                        o_ps[:], ActF.Copy,
                                    )
                                else:
                                    nc.vector.tensor_copy(
                                        o_sb[:, k * NT:(k + 1) * NT], o_ps[:]
                                    )
                            else:
                                nc.tensor.matmul(
                                    o_ps[:],
                                    mT_sb[:, d * 128:(d + 1) * 128],
                                    f_rhs[:, n0:n0 + NT],
                                    start=True,
                                    stop=True,
                                )
                                nc.vector.tensor_add(
                                    o_sb[:, k * NT:(k + 1) * NT],
                                    o_ps[:], xslice(d, n0, NT),
                                )
                        n0 = np4 * 4 * NT
                        (nc.sync if (np4 + d) % 2 == 0 else nc.scalar).dma_start(
                            out[d, :, n0:n0 + 4 * NT], o_sb[:]
                        )

        for it in range(n_iters):
            one_iter(it)

    _split_multiwait(nc)
    return nc


def _get_nc(fast: bool, n_iters: int = 1):
    key = ("nc", fast, n_iters)
    if key not in _CACHE:
        _CACHE[key] = _build_nc(fast, n_iters)
    return _CACHE[key]


def _make_in_maps(x, W_f, W_beta):
    xf = np.ascontiguousarray(x.reshape(B, C, N), dtype=np.float32)
    wft = np.ascontiguousarray(W_f.T.reshape(2, 128, O), dtype=np.float32)
    wbt = np.ascontiguousarray(W_beta.T.reshape(2, 128, C), dtype=np.float32)
    in_maps = []
    for c in range(NCORES):
        b, h = divmod(c, 2)
        shard = np.ascontiguousarray(
            xf[b, :, h * NH:(h + 1) * NH].reshape(2, 128, NH)
        )
        in_maps.append({"xs": shard, "wft": wft, "wbt": wbt})
    return in_maps


def kernel(x: np.ndarray, W_f: np.ndarray, W_beta: np.ndarray) -> np.ndarray:
    global last_results
    fast = os.environ.get("CA_MODE", "safe") == "fast"
    nc = _get_nc(fast)

    in_maps = _make_in_maps(x, W_f, W_beta)
    res = run_bass_kernel_spmd(nc, in_maps, list(range(NCORES)))
    last_results = res

    outv = np.empty((B, C, N), dtype=np.float32)
    for c in range(NCORES):
        b, h = divmod(c, 2)
        outv[b, :, h * NH:(h + 1) * NH] = res.results[c]["out"].reshape(C, NH)
    return outv.reshape(B, C, 128, 128)


# revision 28
# speedup vs baseline: 1.4333x; 1.1167x over previous
"""ChannelAttention kernel for Trainium2 (Bass/Tile), 8-core SPMD.

Reference (per sample b, xf = x[b] as [C=256, N=16384]):
    F  = W_f @ xf                      [50, N]
    S  = softmax(F @ xf^T, axis=C)     [50, 256]
    E  = S^T @ F ; out = W_beta @ E + xf

Key algebraic restructure: out = (W_beta @ S^T) @ F + x = M @ F + x where
M = W_beta @ S^T is a tiny [256, 50] matrix computed once per sample after
softmax — the big E tensor is never materialized.

Sharding: 8 cores = 4 samples x 2 spatial halves (x[b][:, h*8192:(h+1)*8192]).
The only cross-core coupling is the S contraction over N: partial S per
core, AllReduce within pairs [[0,1],[2,3],[4,5],[6,7]] (51 KB), then local.

Per-core dataflow:
  phase 1: F = W_f x; PE-transpose x and F tiles (n-on-partition) and
           accumulate partial S = F x^T in one PSUM bank.
  phase 2: AllReduce S, softmax over the free axis, M^T = S @ W_beta^T.
  phase 3: out = M F + x (residual via DVE add in fp32 mode, or via an
           identity matmul in fp32r mode); DMA out.

Two precision modes (CA_MODE env: "fast" | "safe", default "safe"):
  fast: every matmul in float32r (full PE rate, TF32-like ~7e-4 operand
        rounding). The S logits pick up ~0.1 noise over the 16k-term
        contraction.
  safe: the F and S matmuls (the softmax-logit path) run in true fp32
        (4 cycles/row); the post-softmax out-matmul runs in bf16 (native
        full-rate PE path — measured much faster than fp32r on silicon)
        where rounding is provably harmless (~1e-3 of output scale).

n_iters > 1 repeats the whole dataflow (including DMAs and the collective)
inside one NEFF — used by test.py to measure per-iteration HW time by
differencing, since NTFF profiling is unavailable under axon.
"""

import os
import numpy as np
from contextlib import ExitStack

import concourse.bass as bass
import concourse.tile as tile
from concourse import mybir
from concourse.bass_utils import run_bass_kernel_spmd
from concourse.masks import make_identity

B, C, O = 4, 256, 50
N = 128 * 128            # 16384 spatial positions
NCORES = 8
NH = N // 2              # 8192 per core
NT = 512                 # matmul n-tile
NSUB = 128               # transpose / S sub-tile
XG = 2048                # x DMA group (1 MiB per chunk DMA)
F32 = mybir.dt.float32
F32R = mybir.dt.float32r
BF16 = mybir.dt.bfloat16
ActF = mybir.ActivationFunctionType

_CACHE: dict = {}
last_results = None  # exposes BassKernelResults to test.py

# This walrus build rejects instructions carrying more than one embedded
# semaphore wait ("Too many sync wait commands" in setupSyncWait). After
# Tile finishes sem assignment, hoist excess waits onto InstNoOp
# instructions inserted before the offender on the same engine — engine
# program order makes the split semantically identical.
_MAX_WAITS = 1


def _split_multiwait(nc) -> int:
    n_nops = 0
    for fn in nc.m.functions:
        for blk in fn.blocks:
            out = []
            changed = False
            for inst in list(blk.instructions):
                si = inst.sync_info
                waits = list(si.on_wait) if si is not None and si.on_wait else []
                if len(waits) > _MAX_WAITS:
                    keep = waits[-_MAX_WAITS:]
                    hoist = waits[:-_MAX_WAITS]
                    for i in range(0, len(hoist), _MAX_WAITS):
                        nop = mybir.InstNoOp(name=f"I-waitnop-{n_nops}")
                        n_nops += 1
                        nop.engine = inst.engine
                        nop.sync_info = mybir.SyncInfo(
                            on_wait=hoist[i:i + _MAX_WAITS], on_update=[]
                        )
                        out.append(nop)
                    changed = True
                    inst.sync_info = mybir.SyncInfo(
                        on_wait=keep,
                        on_update=list(si.on_update) if si.on_update else [],
                    )
                out.append(inst)
            if changed:
                blk.instructions = out
    return n_nops


def _build_nc(fast: bool, n_iters: int = 1,
              skip_phase3: bool = False, skip_cc: bool = False) -> bass.Bass:
    """skip_* flags build ablated variants for phase-isolation timing on
    hardware (no NTFF profiler under axon); kernel() never sets them."""
    nc = bass.Bass(num_devices=NCORES)

    xs = nc.dram_tensor("xs", [2, 128, NH], F32, kind="ExternalInput")
    wft = nc.dram_tensor("wft", [2, 128, O], F32, kind="ExternalInput")
    wbt = nc.dram_tensor("wbt", [2, 128, C], F32, kind="ExternalInput")
    out = nc.dram_tensor("out", [2, 128, NH], F32, kind="ExternalOutput")

    n_tiles = NH // NT            # 16
    n_groups = NH // XG           # 4 DMA groups per c-chunk
    subs = NT // NSUB             # 4 sub-tiles per n-tile
    XDT = F32R if fast else F32   # dtype of the softmax-logit path

    with tile.TileContext(nc) as tc, ExitStack() as ctx:
        const = ctx.enter_context(tc.tile_pool(name="const", bufs=1))
        xpool = ctx.enter_context(tc.tile_pool(name="x", bufs=1))
        fpool = ctx.enter_context(tc.tile_pool(name="f", bufs=1))
        stage = ctx.enter_context(tc.tile_pool(name="stage", bufs=4))
        spool = ctx.enter_context(tc.tile_pool(name="smax", bufs=1))
        opool = ctx.enter_context(tc.tile_pool(name="o", bufs=4))
        dram = ctx.enter_context(tc.tile_pool(name="dram", bufs=1, space="DRAM"))

        # weights first (tiny), then x loads can stream
        ident = const.tile([128, 128], F32, tag="ident")
        wft_sb = []
        wbt_sb = []
        for ci in range(2):
            t = const.tile([128, O], XDT, tag=f"wft{ci}")
            (nc.gpsimd if fast else nc.sync).dma_start(t[:], wft[ci])
            wft_sb.append(t)
            t = const.tile([128, C], F32R, tag=f"wbt{ci}")
            nc.gpsimd.dma_start(t[:], wbt[ci])
            wbt_sb.append(t)
        make_identity(nc, ident[:])
        if fast:
            ident_r = const.tile([128, 128], F32R, tag="ident_r")
            nc.vector.tensor_copy(ident_r[:], ident[:])

        def one_iter(it: int):
            # resident x: 2 c-chunks x 4 groups of [128, 2048]; alternate
            # between the two physical HWDGE rings (SP and ACT) so transfer
            # completions overlap instead of serializing on one FIFO
            x_sb = [[None] * n_groups for _ in range(2)]
            for g in range(n_groups):
                for ci in range(2):
                    t = xpool.tile([128, XG], XDT, tag=f"x_{ci}_{g}")
                    if fast:
                        eng = nc.gpsimd
                    else:
                        eng = nc.sync if ci == 0 else nc.scalar
                    eng.dma_start(t[:], xs[ci, :, g * XG:(g + 1) * XG])
                    x_sb[ci][g] = t

            def xslice(ci, n0, w, as_f32=False):
                g, loc = divmod(n0, XG)
                assert loc + w <= XG
                ap = x_sb[ci][g][:, loc:loc + w]
                return ap.bitcast(F32) if (as_f32 and fast) else ap

            f_sb = fpool.tile([O, NH], XDT, tag="F")
            if fast:
                f_rhs = f_sb          # fp32r already
            else:
                # bf16 copy for the post-softmax out-matmul: bf16 runs the
                # guaranteed-native 1 cycle/row PE path, and |out| error from
                # rounding F here is ~3e-4 of output scale (post-softmax,
                # no logit sensitivity)
                f_rhs = fpool.tile([O, NH], BF16, tag="Fr")

            # ---- phase 1: F, x^T, partial S ----
            with tc.tile_pool(name=f"psS{it}", bufs=1, space="PSUM") as psS:
                s_ps = psS.tile([O, C], F32, tag="S")
                with tc.tile_pool(name=f"psF{it}", bufs=2, space="PSUM") as psF, \
                     tc.tile_pool(name=f"psT{it}", bufs=2, space="PSUM") as psT, \
                     tc.tile_pool(name=f"psFT{it}", bufs=2, space="PSUM") as psFT:
                    for nt in range(n_tiles):
                        n0 = nt * NT
                        f_ps = psF.tile([O, NT], F32, tag="f_ps")
                        for ci in range(2):
                            nc.tensor.matmul(
                                f_ps[:],
                                wft_sb[ci][:],
                                xslice(ci, n0, NT),
                                start=(ci == 0),
                                stop=(ci == 1),
                            )
                        nc.scalar.activation(
                            f_sb[:, n0:n0 + NT], f_ps[:], ActF.Copy
                        )
                        if not fast:
                            nc.scalar.activation(
                                f_rhs[:, n0:n0 + NT], f_ps[:], ActF.Copy
                            )

                        # x^T: 8 transposes -> 2 merged PSUM banks -> 2 copies
                        # layout [128, 512] = [s | s+1] x [ci0 | ci1]
                        xT_sb = []
                        for half in range(2):
                            tr_ps = psT.tile([128, 2, C], F32, tag="tr")
                            for s2 in range(2):
                                sn0 = n0 + (half * 2 + s2) * NSUB
                                for ci in range(2):
                                    nc.tensor.transpose(
                                        tr_ps[:, s2,
                                              ci * 128:(ci + 1) * 128],
                                        xslice(ci, sn0, NSUB, as_f32=True),
                                        ident[:],
                                    )
                            xT = stage.tile([128, 2, C], XDT, tag="xT")
                            nc.vector.tensor_copy(xT[:], tr_ps[:])
                            xT_sb.append(xT)

                        # F^T: 4 transposes -> 1 merged PSUM tile -> 1 copy
                        ftr_ps = psFT.tile([128, subs, O], F32, tag="ftr")
                        for s in range(subs):
                            sn0 = n0 + s * NSUB
                            fsrc = f_sb[:, sn0:sn0 + NSUB]
                            nc.tensor.transpose(
                                ftr_ps[:, s],
                                fsrc.bitcast(F32) if fast else fsrc,
                                ident[:O, :O],
                            )
                        fT = stage.tile([128, subs, O], XDT, tag="fT")
                        nc.vector.tensor_copy(fT[:], ftr_ps[:])

                        for s in range(subs):
                            idx = nt * subs + s
                            nc.tensor.matmul(
                                s_ps[:],
                                fT[:, s],
                                xT_sb[s // 2][:, s % 2],
                                start=(idx == 0),
                                stop=(idx == n_tiles * subs - 1),
                            )

                # ---- phase 2: AllReduce partial S + softmax + M ----
                s_part = spool.tile([O, C], F32, tag="s_part")
                nc.vector.tensor_copy(s_part[:], s_ps[:])

            cc_in = dram.tile([O, C], F32, tag="cc_in")
            cc_out = dram.tile([O, C], F32, tag="cc_out")
            nc.sync.dma_start(cc_in[:], s_part[:])
            if skip_cc:
                nc.sync.dma_start(cc_out[:], cc_in[:])
            else:
                nc.gpsimd.collective_compute(
                    "AllReduce",
                    mybir.AluOpType.add,
                    replica_groups=[[0, 1], [2, 3], [4, 5], [6, 7]],
                    ins=[cc_in.opt()],
                    outs=[cc_out.opt()],
                )
            s_full = spool.tile([O, C], F32, tag="s_full")
            nc.sync.dma_start(s_full[:], cc_out[:])

            mx = spool.tile([O, 1], F32, tag="mx")
            nc.vector.tensor_reduce(
                mx[:], s_full[:], axis=mybir.AxisListType.X,
                op=mybir.AluOpType.max,
            )
            nmx = spool.tile([O, 1], F32, tag="nmx")
            nc.vector.tensor_scalar_mul(nmx[:], mx[:], -1.0)
            p_exp = spool.tile([O, C], F32, tag="p_exp")
            ssum = spool.tile([O, 1], F32, tag="ssum")
            nc.scalar.activation(
                p_exp[:], s_full[:], ActF.Exp, bias=nmx[:], accum_out=ssum[:]
            )
            rsum = spool.tile([O, 1], F32, tag="rsum")
            nc.vector.reciprocal(rsum[:], ssum[:])
            p_norm = spool.tile([O, C], F32, tag="p_norm")
            nc.vector.tensor_scalar_mul(p_norm[:], p_exp[:], rsum[:])

            # ---- phase 3: out = M F + x, two n-tiles per store DMA ----
            # psO opens BEFORE psM so the PSUM stack gives psO banks that
            # don't wait on psM's release: the residual identity-matmuls
            # (which don't depend on M) can then fill o_ps banks during the
            # collective/softmax bubble. psM uses a single sequentially
            # reused bank (6 + 1 <= 8).
            with tc.tile_pool(name=f"psO{it}", bufs=6, space="PSUM") as psO, \
                 tc.tile_pool(name=f"psM{it}", bufs=1, space="PSUM") as psM:
                # M^T = S @ W_beta^T  [50, 256]
                mT_sb = spool.tile([O, C], F32R if fast else BF16, tag="mT")
                st_sb = []
                for ci in range(2):
                    st_ps = psM.tile([128, O], F32, tag="m_seq")
                    nc.tensor.transpose(
                        st_ps[:], p_norm[:, ci * 128:(ci + 1) * 128],
                        ident[:O, :O],
                    )
                    t = spool.tile([128, O], F32R, tag=f"st_sb{ci}")
                    nc.vector.tensor_copy(t[:], st_ps[:])
                    st_sb.append(t)
                m_ps = psM.tile([O, C], F32, tag="m_seq")
                for ci in range(2):
                    nc.tensor.matmul(
                        m_ps[:],
                        st_sb[ci][:],
                        wbt_sb[ci][:],
                        start=(ci == 0),
                        stop=(ci == 1),
                    )
                nc.vector.tensor_copy(mT_sb[:], m_ps[:])

                if skip_phase3:
                    return
                for np4 in range(n_tiles // 4):
                    for d in range(2):
                        # 4 n-tiles per 1 MiB store; alternate HWDGE rings
                        o_sb = opool.tile([128, 4 * NT], F32, tag="o_sb")
                        for k in range(4):
                            nt = np4 * 4 + k
                            n0 = nt * NT
                            o_ps = psO.tile([128, NT], F32, tag="o_ps")
                            if fast:
                                nc.tensor.matmul(
                                    o_ps[:],
                                    mT_sb[:, d * 128:(d + 1) * 128],
                                    f_rhs[:, n0:n0 + NT],
                                    start=True,
                                    stop=False,
                                )
                                nc.tensor.matmul(
                                    o_ps[:],
                                    ident_r[:],
                                    xslice(d, n0, NT),
                                    start=False,
                                    stop=True,
                                )
                                if nt % 2 == 0:
                                    nc.scalar.activation(
                                        o_sb[:, k * NT:(k + 1) * NT],
                                        o_ps[:], ActF.Copy,
                                    )
                                else:
                                    nc.vector.tensor_copy(
                                        o_sb[:, k * NT:(k + 1) * NT], o_ps[:]
                                    )
                            else:
                                nc.tensor.matmul(
                                    o_ps[:],
                                    mT_sb[:, d * 128:(d + 1) * 128],
                                    f_rhs[:, n0:n0 + NT],
                                    start=True,
                                    stop=True,
                                )
                                osl = o_sb[:, k * NT:(k + 1) * NT]
                                if nt % 2 == 0:
                                    # split residual work across engines:
                                    # ACT evacuates PSUM, DVE adds x with
                                    # both operands in SBUF (2x mode)
                                    nc.scalar.activation(
                                        osl, o_ps[:], ActF.Copy
                                    )
                                    nc.vector.tensor_add(
                                        osl, osl, xslice(d, n0, NT)
                                    )
                                else:
                                    nc.vector.tensor_add(
                                        osl, o_ps[:], xslice(d, n0, NT)
                                    )
                        n0 = np4 * 4 * NT
                        (nc.sync if (np4 + d) % 2 == 0 else nc.scalar).dma_start(
                            out[d, :, n0:n0 + 4 * NT], o_sb[:]
                        )

        for it in range(n_iters):
            one_iter(it)

    _split_multiwait(nc)
    return nc


def _get_nc(fast: bool, n_iters: int = 1):
    key = ("nc", fast, n_iters)
    if key not in _CACHE:
        _CACHE[key] = _build_nc(fast, n_iters)
    return _CACHE[key]


def _make_in_maps(x, W_f, W_beta):
    xf = np.ascontiguousarray(x.reshape(B, C, N), dtype=np.float32)
    wft = np.ascontiguousarray(W_f.T.reshape(2, 128, O), dtype=np.float32)
    wbt = np.ascontiguousarray(W_beta.T.reshape(2, 128, C), dtype=np.float32)
    in_maps = []
    for c in range(NCORES):
        b, h = divmod(c, 2)
        shard = np.ascontiguousarray(
            xf[b, :, h * NH:(h + 1) * NH].reshape(2, 128, NH)
        )
        in_maps.append({"xs": shard, "wft": wft, "wbt": wbt})
    return in_maps


def kernel(x: np.ndarray, W_f: np.ndarray, W_beta: np.ndarray) -> np.ndarray:
    global last_results
    fast = os.environ.get("CA_MODE", "safe") == "fast"
    nc = _get_nc(fast)

    in_maps = _make_in_maps(x, W_f, W_beta)
    res = run_bass_kernel_spmd(nc, in_maps, list(range(NCORES)))
    last_results = res

    outv = np.empty((B, C, N), dtype=np.float32)
    for c in range(NCORES):
        b, h = divmod(c, 2)
        outv[b, :, h * NH:(h + 1) * NH] = res.results[c]["out"].reshape(C, NH)
    return outv.reshape(B, C, 128, 128)
